# revision 11
# baseline (speedup 1.0000x reference)
"""Trainium2 Bass kernel for nn_AttentionGated (gated-attention MIL pooling).

Math (per batch b):
  h = relu(x @ Wf + bf)            [B, N, L]    L=512
  a = tanh(h @ Wa + ba)            [B, N, D]    D=384
  g = sigmoid(h @ Wb + bb)         [B, N, D]
  s = (a*g) @ Wc + bc              [B, N, 1]
  A = softmax(s over N)            [B, 1, N]
  pooled = A @ h                   [B, 1, L]
  logits = pooled @ Wcls + bcls    [B, 2]

Strategy: shard N across 8 cores (5000 instances/core/batch). Each core
streams its x shard (fp32, cast to bf16 in-flight by SWDGE DMA), DMA-
transposes to feature-major, runs the fused pipeline in bf16 on PE/ACT/DVE,
accumulates unnormalized pooled numerators P_b = sum_n exp(s_n) h_n and
denominators S_b = sum_n exp(s_n) (scores are tightly bounded, so no max
subtraction is needed), then a single AllReduce(add) over [P|S] combines
cores and every core computes the final logits.

sigmoid(x) = 0.5*tanh(x/2) + 0.5 so ACT only needs the exp_and_others
table set (tanh + exp) -> no per-group table switches.
"""

import os
import sys

for _p in ("/opt/trn_rl_repo", "/root/.axon_site/_ro/trn_rl_repo"):
    if os.path.isdir(_p) and _p not in sys.path:
        sys.path.insert(0, _p)

from contextlib import ExitStack

import ml_dtypes
import numpy as np

import concourse.bass as bass
import concourse.mybir as mybir
import concourse.tile as tile
from concourse import bacc
from concourse.bass_utils import run_bass_kernel_spmd

F32 = mybir.dt.float32
BF16 = mybir.dt.bfloat16

B, N, D_IN, L, D, NCLS = 2, 40000, 1024, 512, 384, 2
CORES = 8
NS = N // CORES          # 5000 instances per core per batch
G = 1024                 # rows per processing group
GB = 5                   # groups per batch (4 full + 1 tail)
TAIL = NS - (GB - 1) * G  # 904 real rows in the tail group
NGROUPS = B * GB         # 10
KT_F = D_IN // 128       # 8  k-tiles for Wf
MT_L = L // 128          # 4  l-tiles
KT_L = L // 128          # 4  k-tiles for Wa/Wb
MT_D = D // 128          # 3  d-tiles
NEG_FILL = -40.0         # exp(-40) ~ 4e-18: kills padded rows


def build_kernel(collective=True):
    nc = bacc.Bacc(
        "TRN2",
        target_bir_lowering=False,
        debug=False,
        num_devices=CORES,
    )

    x_t = nc.dram_tensor("x", [B, NS, D_IN], F32, kind="ExternalInput")
    wf_t = nc.dram_tensor("wf", [D_IN, L], BF16, kind="ExternalInput")
    wa_t = nc.dram_tensor("wa", [L, D], BF16, kind="ExternalInput")
    wb_t = nc.dram_tensor("wb", [L, D], BF16, kind="ExternalInput")
    wc_t = nc.dram_tensor("wc", [D, 1], BF16, kind="ExternalInput")
    wcls_t = nc.dram_tensor("wcls", [L, NCLS], F32, kind="ExternalInput")
    bf_t = nc.dram_tensor("bf", [L], F32, kind="ExternalInput")
    ba_t = nc.dram_tensor("ba", [D], F32, kind="ExternalInput")
    bbh_t = nc.dram_tensor("bbh", [D], F32, kind="ExternalInput")  # bb/2
    bc_t = nc.dram_tensor("bc", [1], F32, kind="ExternalInput")
    bcls_t = nc.dram_tensor("bcls", [NCLS], F32, kind="ExternalInput")
    if collective:
        out_t = nc.dram_tensor("out", [B, NCLS], F32, kind="ExternalOutput")
    else:
        out_t = nc.dram_tensor("out_ps", [B, L + 1], F32, kind="ExternalOutput")

    with tile.TileContext(nc) as tc:
        _body(tc, nc, x_t, wf_t, wa_t, wb_t, wc_t, wcls_t,
              bf_t, ba_t, bbh_t, bc_t, bcls_t, out_t, collective)

    nc.compile()
    return nc


def _body(tc, nc, x_t, wf_t, wa_t, wb_t, wc_t, wcls_t,
          bf_t, ba_t, bbh_t, bc_t, bcls_t, out_t, collective=True):
    AL = mybir.AluOpType

    ctx = ExitStack()
    wpool = ctx.enter_context(tc.tile_pool(name="weights", bufs=1))
    fpool = ctx.enter_context(tc.tile_pool(name="final", bufs=1))
    stage_pool = ctx.enter_context(tc.tile_pool(name="stage", bufs=3))
    xnat_pool = ctx.enter_context(tc.tile_pool(name="xnat", bufs=2))
    xt_pool = ctx.enter_context(tc.tile_pool(name="xt", bufs=2))
    h_pool = ctx.enter_context(tc.tile_pool(name="h", bufs=2))
    ag_pool = ctx.enter_context(tc.tile_pool(name="ag", bufs=2))
    e_pool = ctx.enter_context(tc.tile_pool(name="e", bufs=2))
    s_pool = ctx.enter_context(tc.tile_pool(name="s", bufs=2))
    psum_mm = ctx.enter_context(tc.tile_pool(name="psum_mm", bufs=3, space="PSUM"))
    psum_vec = ctx.enter_context(tc.tile_pool(name="psum_vec", bufs=1, space="PSUM"))
    dram_pool = ctx.enter_context(tc.tile_pool(name="dram", bufs=1, space="DRAM"))

    # ---- weights / constants (loaded once) ----
    wf_sb = wpool.tile([128, KT_F, L], BF16)
    nc.sync.dma_start(wf_sb[:], wf_t.ap().rearrange("(kt p) l -> p kt l", p=128))
    wa_sb = wpool.tile([128, KT_L, D], BF16)
    nc.sync.dma_start(wa_sb[:], wa_t.ap().rearrange("(kt p) d -> p kt d", p=128))
    wb_sb = wpool.tile([128, KT_L, D], BF16)
    nc.sync.dma_start(wb_sb[:], wb_t.ap().rearrange("(kt p) d -> p kt d", p=128))
    wc_sb = wpool.tile([128, MT_D, 1], BF16)
    nc.sync.dma_start(wc_sb[:], wc_t.ap().rearrange("(kt p) o -> p kt o", p=128))
    wcls_sb = wpool.tile([128, MT_L, NCLS], F32)
    nc.sync.dma_start(wcls_sb[:], wcls_t.ap().rearrange("(kt p) c -> p kt c", p=128))

    bf_sb = wpool.tile([128, MT_L], F32)
    nc.sync.dma_start(bf_sb[:], bf_t.ap().rearrange("(m p) -> p m", p=128))
    ba_sb = wpool.tile([128, MT_D], F32)
    nc.sync.dma_start(ba_sb[:], ba_t.ap().rearrange("(m p) -> p m", p=128))
    bbh_sb = wpool.tile([128, MT_D], F32)
    nc.sync.dma_start(bbh_sb[:], bbh_t.ap().rearrange("(m p) -> p m", p=128))
    bc_sb = wpool.tile([1, 1], F32)
    nc.sync.dma_start(bc_sb[:], bc_t.ap()[None, :])
    bcls_sb = wpool.tile([1, NCLS], F32)
    nc.sync.dma_start(bcls_sb[:], bcls_t.ap()[None, :])

    ones_sb = wpool.tile([1, 128], BF16)
    nc.vector.memset(ones_sb[:], 1.0)

    # persistent accumulators
    P_parts = fpool.tile([128, MT_L * NGROUPS], F32)   # col = m*NGROUPS + g
    S_parts = fpool.tile([128, NGROUPS], F32)
    junk = fpool.tile([128, G], BF16)                  # TTR product sink

    # ---- main pipeline over groups ----
    for b in range(B):
        for gi in range(GB):
            g = b * GB + gi
            r0 = gi * G
            rows = G if gi < GB - 1 else TAIL

            # -- load x rows fp32 (HWDGE), cast fp32->bf16 on idle GPSIMD --
            # (SWDGE dtype-cast DMA crashes NRT on this runtime, so the
            # cast runs as a gpsimd tensor_copy instead.)
            HS = G // 256                            # 4 subtiles per half
            x_nat = xnat_pool.tile([128, G // 128, D_IN], BF16, name="x_nat")
            for hf in range(2):
                st = stage_pool.tile([128, HS, D_IN], F32, name="st")
                hr0 = r0 + hf * HS * 128             # first row of this half
                hrows = min(rows - hf * HS * 128, HS * 128)
                if hrows == HS * 128:
                    nc.sync.dma_start(
                        st[:],
                        x_t.ap()[b, hr0:hr0 + HS * 128, :]
                        .rearrange("(s p) d -> p s d", p=128),
                    )
                else:
                    full_s = hrows // 128            # full subtiles in half
                    rem = hrows - full_s * 128       # leftover rows
                    nc.vector.memset(st[:, full_s:, :], 0.0)
                    if full_s:
                        nc.sync.dma_start(
                            st[:, :full_s, :],
                            x_t.ap()[b, hr0:hr0 + full_s * 128, :]
                            .rearrange("(s p) d -> p s d", p=128),
                        )
                    if rem:
                        nc.sync.dma_start(
                            st[:rem, full_s, :],
                            x_t.ap()[b, hr0 + full_s * 128:hr0 + hrows, :],
                        )
                nc.gpsimd.tensor_copy(x_nat[:, hf * HS:(hf + 1) * HS, :], st[:])

            # -- transpose to feature-major: xt[pi, s, kt, p] = x[r0+128s+p, 128kt+pi]
            xt = xt_pool.tile([128, G // 128, KT_F, 128], BF16, name="xt")
            nc.sync.dma_start(xt[:], x_nat[:], transpose=True)

            # -- h^T = relu(Wf^T x^T + bf):  4 l-tiles of [128, G]
            h_sb = h_pool.tile([128, MT_L, G], BF16, name="h_sb")
            for m in range(MT_L):
                h_ps = psum_mm.tile([128, G], F32, name="h_ps", tag="mm")
                for kt in range(KT_F):
                    for c in range(G // 512):
                        nc.tensor.matmul(
                            h_ps[:, c * 512:(c + 1) * 512],
                            lhsT=wf_sb[:, kt, m * 128:(m + 1) * 128],
                            rhs=xt[:, 4 * c:4 * c + 4, kt, :],
                            start=(kt == 0),
                            stop=(kt == KT_F - 1),
                        )
                # relu + bias, evacuate PSUM -> SBUF bf16 (DVE)
                nc.vector.tensor_scalar(
                    h_sb[:, m, :], h_ps[:],
                    bf_sb[:, m:m + 1], 0.0, AL.add, AL.max,
                )

            # -- a^T = tanh(Wa^T h^T + ba); gt = tanh((Wb^T h^T + bb)/2)
            a_sb = ag_pool.tile([128, MT_D, G], BF16, name="a_sb")
            g_sb = ag_pool.tile([128, MT_D, G], BF16, name="g_sb")
            for m in range(MT_D):
                a_ps = psum_mm.tile([128, G], F32, name="a_ps", tag="mm")
                for kt in range(KT_L):
                    for c in range(G // 512):
                        nc.tensor.matmul(
                            a_ps[:, c * 512:(c + 1) * 512],
                            lhsT=wa_sb[:, kt, m * 128:(m + 1) * 128],
                            rhs=h_sb[:, kt, c * 512:(c + 1) * 512],
                            start=(kt == 0),
                            stop=(kt == KT_L - 1),
                        )
                nc.scalar.activation(
                    a_sb[:, m, :], a_ps[:],
                    mybir.ActivationFunctionType.Tanh,
                    bias=ba_sb[:, m:m + 1],
                )
            for m in range(MT_D):
                g_ps = psum_mm.tile([128, G], F32, name="g_ps", tag="mm")
                for kt in range(KT_L):
                    for c in range(G // 512):
                        nc.tensor.matmul(
                            g_ps[:, c * 512:(c + 1) * 512],
                            lhsT=wb_sb[:, kt, m * 128:(m + 1) * 128],
                            rhs=h_sb[:, kt, c * 512:(c + 1) * 512],
                            start=(kt == 0),
                            stop=(kt == KT_L - 1),
                        )
                # gt = tanh(0.5*z + bb/2); sigmoid(z) = 0.5*gt + 0.5
                nc.scalar.activation(
                    g_sb[:, m, :], g_ps[:],
                    mybir.ActivationFunctionType.Tanh,
                    bias=bbh_sb[:, m:m + 1], scale=0.5,
                )
                nc.vector.tensor_scalar(
                    g_sb[:, m, :], g_sb[:, m, :], 0.5, 0.5, AL.mult, AL.add,
                )
            # ag = a * g (in place into a_sb)
            for m in range(MT_D):
                nc.vector.tensor_tensor(
                    a_sb[:, m, :], a_sb[:, m, :], g_sb[:, m, :], AL.mult,
                )

            # -- s = Wc^T (a*g):  [1, G]
            s_ps = psum_vec.tile([1, G], F32, name="s_ps", tag="vec")
            for kt in range(MT_D):
                for c in range(G // 512):
                    nc.tensor.matmul(
                        s_ps[:, c * 512:(c + 1) * 512],
                        lhsT=wc_sb[:, kt, :],
                        rhs=a_sb[:, kt, c * 512:(c + 1) * 512],
                        start=(kt == 0),
                        stop=(kt == MT_D - 1),
                    )
            s_sb = s_pool.tile([1, G], BF16, name="s_sb")
            nc.vector.tensor_scalar(
                s_sb[:], s_ps[:], bc_sb[0:1, 0:1], None, AL.add,
            )
            if rows < G:
                nc.vector.memset(s_sb[:, rows:], NEG_FILL)

            # -- broadcast s to all partitions, then e = exp(s) with running sum
            sbc_ps = psum_vec.tile([128, G], F32, name="sbc_ps", tag="vec")
            for c in range(G // 512):
                nc.tensor.matmul(
                    sbc_ps[:, c * 512:(c + 1) * 512],
                    lhsT=ones_sb[:],
                    rhs=s_sb[:, c * 512:(c + 1) * 512],
                    start=True, stop=True,
                )
            e_sb = e_pool.tile([128, G], BF16, name="e_sb")
            nc.scalar.activation(
                e_sb[:], sbc_ps[:],
                mybir.ActivationFunctionType.Exp,
                accum_out=S_parts[:, g:g + 1],
            )

            # -- pooled numerator partials: P[m] += sum_r e_r * h^T[m][:, r]
            # (tensor_tensor_reduce is a custom DVE op this runtime lacks,
            # so use plain mult + reduce.)
            for m in range(MT_L):
                nc.vector.tensor_tensor(
                    junk[:], h_sb[:, m, :], e_sb[:], AL.mult,
                )
                nc.vector.tensor_reduce(
                    P_parts[:, m * NGROUPS + g:m * NGROUPS + g + 1],
                    junk[:],
                    axis=mybir.AxisListType.X, op=AL.add,
                )

    # ---- reduce partials, all-reduce across cores, final logits ----
    P_final = fpool.tile([128, MT_L * B], F32)   # col = m*B + b
    S_final = fpool.tile([128, B], F32)
    for b in range(B):
        for m in range(MT_L):
            nc.vector.tensor_reduce(
                P_final[:, m * B + b:m * B + b + 1],
                P_parts[:, m * NGROUPS + b * GB:m * NGROUPS + (b + 1) * GB],
                axis=mybir.AxisListType.X, op=AL.add,
            )
        nc.vector.tensor_reduce(
            S_final[:, b:b + 1],
            S_parts[:, b * GB:(b + 1) * GB],
            axis=mybir.AxisListType.X, op=AL.add,
        )

    if not collective:
        for b in range(B):
            nc.sync.dma_start(
                out_t.ap()[b, 0:L].rearrange("(m p) -> p m", p=128),
                P_final[:, b::B],
            )
            nc.sync.dma_start(out_t.ap()[b, L:L + 1], S_final[0:1, b:b + 1])
        ctx.close()
        return

    cc_in = dram_pool.tile([B, L + 1], F32, name="cc_in")
    cc_out = dram_pool.tile([B, L + 1], F32, name="cc_out", addr_space="Shared")
    for b in range(B):
        nc.sync.dma_start(
            cc_in[b, 0:L].rearrange("(m p) -> p m", p=128),
            P_final[:, b::B],
        )
        nc.sync.dma_start(cc_in[b, L:L + 1], S_final[0:1, b:b + 1])

    nc.gpsimd.collective_compute(
        "AllReduce",
        AL.add,
        replica_groups=[list(range(CORES))],
        ins=[cc_in[:, :]],
        outs=[cc_out[:, :]],
    )

    pooledT = fpool.tile([128, MT_L, B], F32)   # numerators, feature-major
    for b in range(B):
        nc.sync.dma_start(
            pooledT[:, :, b],
            cc_out[b, 0:L].rearrange("(m p) -> p m", p=128),
        )
    S_col = fpool.tile([B, 1], F32)
    nc.sync.dma_start(S_col[:], cc_out[:, L:L + 1])
    S_row = fpool.tile([1, B], F32)
    nc.sync.dma_start(S_row[:], cc_out[:, L:L + 1].rearrange("b o -> o b"))
    S_inv = fpool.tile([B, 1], F32)
    nc.vector.reciprocal(S_inv[:], S_col[:])

    # logits[b, c] = (sum_l P[b, l] Wcls[l, c] + S_b * bcls[c]) / S_b
    lg_ps = psum_vec.tile([B, NCLS], F32, name="lg_ps", tag="vec")
    for kt in range(MT_L):
        nc.tensor.matmul(
            lg_ps[:],
            lhsT=pooledT[:, kt, :],
            rhs=wcls_sb[:, kt, :],
            start=(kt == 0), stop=False,
        )
    nc.tensor.matmul(
        lg_ps[:], lhsT=S_row[:], rhs=bcls_sb[:], start=False, stop=True,
    )
    lg_sb = fpool.tile([B, NCLS], F32)
    nc.vector.tensor_scalar(lg_sb[:], lg_ps[:], S_inv[:], None, AL.mult)
    nc.sync.dma_start(out_t.ap()[:, :], lg_sb[:])
    ctx.close()


_NC_CACHE = None


def _get_nc():
    global _NC_CACHE
    if _NC_CACHE is None:
        _NC_CACHE = build_kernel()
    return _NC_CACHE


def make_in_maps(inputs):
    x = np.asarray(inputs["x"], dtype=np.float32)
    wf = np.asarray(inputs["Wf"], dtype=np.float32).astype(ml_dtypes.bfloat16)
    wa = np.asarray(inputs["Wa"], dtype=np.float32).astype(ml_dtypes.bfloat16)
    wb = np.asarray(inputs["Wb"], dtype=np.float32).astype(ml_dtypes.bfloat16)
    wc = np.asarray(inputs["Wc"], dtype=np.float32).astype(ml_dtypes.bfloat16)
    wcls = np.asarray(inputs["Wcls"], dtype=np.float32)
    bf = np.asarray(inputs["bf"], dtype=np.float32)
    ba = np.asarray(inputs["ba"], dtype=np.float32)
    bbh = np.asarray(inputs["bb"], dtype=np.float32) * 0.5
    bc = np.asarray(inputs["bc"], dtype=np.float32).reshape([1])
    bcls = np.asarray(inputs["bcls"], dtype=np.float32)

    shared = {
        "wf": wf, "wa": wa, "wb": wb, "wc": wc, "wcls": wcls,
        "bf": bf, "ba": ba, "bbh": bbh, "bc": bc, "bcls": bcls,
    }
    in_maps = []
    for c in range(CORES):
        shard = np.ascontiguousarray(x[:, c * NS:(c + 1) * NS, :])
        in_maps.append({"x": shard, **shared})
    return in_maps


def kernel(**inputs) -> np.ndarray:
    nc = _get_nc()
    in_maps = make_in_maps(inputs)
    res = run_bass_kernel_spmd(nc, in_maps, core_ids=list(range(CORES)))
    return np.asarray(res.results[0]["out"], dtype=np.float32)


if __name__ == "__main__":
    nc = build_kernel()
    print("build OK, instructions:", len(nc.m.functions[0].instructions)
          if hasattr(nc.m.functions[0], "instructions") else "?")


# revision 12
# speedup vs baseline: 1.2051x; 1.2051x over previous
"""Trainium2 Bass kernel for nn_AttentionGated (gated-attention MIL pooling).

Math (per batch b):
  h = relu(x @ Wf + bf)            [B, N, L]    L=512
  a = tanh(h @ Wa + ba)            [B, N, D]    D=384
  g = sigmoid(h @ Wb + bb)         [B, N, D]
  s = (a*g) @ Wc + bc              [B, N, 1]
  A = softmax(s over N)            [B, 1, N]
  pooled = A @ h                   [B, 1, L]
  logits = pooled @ Wcls + bcls    [B, 2]

Strategy: shard N across 8 cores (5000 instances/core/batch). Each core
streams its x shard (fp32, cast to bf16 in-flight by SWDGE DMA), DMA-
transposes to feature-major, runs the fused pipeline in bf16 on PE/ACT/DVE,
accumulates unnormalized pooled numerators P_b = sum_n exp(s_n) h_n and
denominators S_b = sum_n exp(s_n) (scores are tightly bounded, so no max
subtraction is needed), then a single AllReduce(add) over [P|S] combines
cores and every core computes the final logits.

sigmoid(x) = 0.5*tanh(x/2) + 0.5 so ACT only needs the exp_and_others
table set (tanh + exp) -> no per-group table switches.
"""

import os
import sys

for _p in ("/opt/trn_rl_repo", "/root/.axon_site/_ro/trn_rl_repo"):
    if os.path.isdir(_p) and _p not in sys.path:
        sys.path.insert(0, _p)

from contextlib import ExitStack

import ml_dtypes
import numpy as np

import concourse.bass as bass
import concourse.mybir as mybir
import concourse.tile as tile
from concourse import bacc
from concourse.bass_utils import run_bass_kernel_spmd

F32 = mybir.dt.float32
BF16 = mybir.dt.bfloat16

B, N, D_IN, L, D, NCLS = 2, 40000, 1024, 512, 384, 2
CORES = 8
NS = N // CORES          # 5000 instances per core per batch
G = 1024                 # rows per processing group
GB = 5                   # groups per batch (4 full + 1 tail)
TAIL = NS - (GB - 1) * G  # 904 real rows in the tail group
NGROUPS = B * GB         # 10
KT_F = D_IN // 128       # 8  k-tiles for Wf
MT_L = L // 128          # 4  l-tiles
KT_L = L // 128          # 4  k-tiles for Wa/Wb
MT_D = D // 128          # 3  d-tiles
NEG_FILL = -40.0         # exp(-40) ~ 4e-18: kills padded rows


def build_kernel(collective=True):
    nc = bacc.Bacc(
        "TRN2",
        target_bir_lowering=False,
        debug=False,
        num_devices=CORES,
    )

    x_t = nc.dram_tensor("x", [B, NS, D_IN], F32, kind="ExternalInput")
    wf_t = nc.dram_tensor("wf", [D_IN, L], BF16, kind="ExternalInput")
    wa_t = nc.dram_tensor("wa", [L, D], BF16, kind="ExternalInput")
    wb_t = nc.dram_tensor("wb", [L, D], BF16, kind="ExternalInput")
    wc_t = nc.dram_tensor("wc", [D, 1], BF16, kind="ExternalInput")
    wcls_t = nc.dram_tensor("wcls", [L, NCLS], F32, kind="ExternalInput")
    bf_t = nc.dram_tensor("bf", [L], F32, kind="ExternalInput")
    ba_t = nc.dram_tensor("ba", [D], F32, kind="ExternalInput")
    bbh_t = nc.dram_tensor("bbh", [D], F32, kind="ExternalInput")  # bb/2
    bc_t = nc.dram_tensor("bc", [1], F32, kind="ExternalInput")
    bcls_t = nc.dram_tensor("bcls", [NCLS], F32, kind="ExternalInput")
    if collective:
        out_t = nc.dram_tensor("out", [B, NCLS], F32, kind="ExternalOutput")
    else:
        out_t = nc.dram_tensor("out_ps", [B, L + 1], F32, kind="ExternalOutput")

    with tile.TileContext(nc) as tc:
        _body(tc, nc, x_t, wf_t, wa_t, wb_t, wc_t, wcls_t,
              bf_t, ba_t, bbh_t, bc_t, bcls_t, out_t, collective)

    nc.compile()
    return nc


def _body(tc, nc, x_t, wf_t, wa_t, wb_t, wc_t, wcls_t,
          bf_t, ba_t, bbh_t, bc_t, bcls_t, out_t, collective=True):
    AL = mybir.AluOpType

    ctx = ExitStack()
    wpool = ctx.enter_context(tc.tile_pool(name="weights", bufs=1))
    fpool = ctx.enter_context(tc.tile_pool(name="final", bufs=1))
    stage_pool = ctx.enter_context(tc.tile_pool(name="stage", bufs=3))
    xnat_pool = ctx.enter_context(tc.tile_pool(name="xnat", bufs=2))
    xt_pool = ctx.enter_context(tc.tile_pool(name="xt", bufs=2))
    h_pool = ctx.enter_context(tc.tile_pool(name="h", bufs=2))
    ag_pool = ctx.enter_context(tc.tile_pool(name="ag", bufs=2))
    e_pool = ctx.enter_context(tc.tile_pool(name="e", bufs=2))
    s_pool = ctx.enter_context(tc.tile_pool(name="s", bufs=2))
    psum_mm = ctx.enter_context(tc.tile_pool(name="psum_mm", bufs=3, space="PSUM"))
    psum_vec = ctx.enter_context(tc.tile_pool(name="psum_vec", bufs=1, space="PSUM"))
    dram_pool = ctx.enter_context(tc.tile_pool(name="dram", bufs=1, space="DRAM"))

    # ---- weights / constants (loaded once) ----
    wf_sb = wpool.tile([128, KT_F, L], BF16)
    nc.sync.dma_start(wf_sb[:], wf_t.ap().rearrange("(kt p) l -> p kt l", p=128))
    wa_sb = wpool.tile([128, KT_L, D], BF16)
    nc.sync.dma_start(wa_sb[:], wa_t.ap().rearrange("(kt p) d -> p kt d", p=128))
    wb_sb = wpool.tile([128, KT_L, D], BF16)
    nc.sync.dma_start(wb_sb[:], wb_t.ap().rearrange("(kt p) d -> p kt d", p=128))
    wc_sb = wpool.tile([128, MT_D, 1], BF16)
    nc.sync.dma_start(wc_sb[:], wc_t.ap().rearrange("(kt p) o -> p kt o", p=128))
    wcls_sb = wpool.tile([128, MT_L, NCLS], F32)
    nc.sync.dma_start(wcls_sb[:], wcls_t.ap().rearrange("(kt p) c -> p kt c", p=128))

    bf_sb = wpool.tile([128, MT_L], F32)
    nc.sync.dma_start(bf_sb[:], bf_t.ap().rearrange("(m p) -> p m", p=128))
    ba_sb = wpool.tile([128, MT_D], F32)
    nc.sync.dma_start(ba_sb[:], ba_t.ap().rearrange("(m p) -> p m", p=128))
    bbh_sb = wpool.tile([128, MT_D], F32)
    nc.sync.dma_start(bbh_sb[:], bbh_t.ap().rearrange("(m p) -> p m", p=128))
    bc_sb = wpool.tile([1, 1], F32)
    nc.sync.dma_start(bc_sb[:], bc_t.ap()[None, :])
    bcls_sb = wpool.tile([1, NCLS], F32)
    nc.sync.dma_start(bcls_sb[:], bcls_t.ap()[None, :])

    ones_sb = wpool.tile([1, 128], BF16)
    nc.vector.memset(ones_sb[:], 1.0)

    # persistent accumulators
    P_parts = fpool.tile([128, MT_L * NGROUPS], F32)   # col = m*NGROUPS + g
    S_parts = fpool.tile([128, NGROUPS], F32)
    junk = fpool.tile([128, G], BF16)                  # TTR product sink

    # ---- main pipeline over groups ----
    for b in range(B):
        for gi in range(GB):
            g = b * GB + gi
            r0 = gi * G
            rows = G if gi < GB - 1 else TAIL

            # -- load x rows fp32 (HWDGE), cast fp32->bf16 on idle GPSIMD --
            # (SWDGE dtype-cast DMA crashes NRT on this runtime, so the
            # cast runs as a gpsimd tensor_copy instead.)
            HS = G // 256                            # 4 subtiles per half
            x_nat = xnat_pool.tile([128, G // 128, D_IN], BF16, name="x_nat")
            for hf in range(2):
                st = stage_pool.tile([128, HS, D_IN], F32, name="st")
                hr0 = r0 + hf * HS * 128             # first row of this half
                hrows = min(rows - hf * HS * 128, HS * 128)
                if hrows == HS * 128:
                    nc.sync.dma_start(
                        st[:],
                        x_t.ap()[b, hr0:hr0 + HS * 128, :]
                        .rearrange("(s p) d -> p s d", p=128),
                    )
                else:
                    full_s = hrows // 128            # full subtiles in half
                    rem = hrows - full_s * 128       # leftover rows
                    nc.vector.memset(st[:, full_s:, :], 0.0)
                    if full_s:
                        nc.sync.dma_start(
                            st[:, :full_s, :],
                            x_t.ap()[b, hr0:hr0 + full_s * 128, :]
                            .rearrange("(s p) d -> p s d", p=128),
                        )
                    if rem:
                        nc.sync.dma_start(
                            st[:rem, full_s, :],
                            x_t.ap()[b, hr0 + full_s * 128:hr0 + hrows, :],
                        )
                # split the cast between ACT (own SBUF ports) and DVE
                # (2x two-port mode) -- gpsimd's software copy is ~4x
                # slower and its SBUF port use stalls concurrent DVE ops.
                if hf == 0:
                    nc.scalar.copy(x_nat[:, hf * HS:(hf + 1) * HS, :], st[:])
                else:
                    nc.vector.tensor_copy(x_nat[:, hf * HS:(hf + 1) * HS, :], st[:])

            # -- transpose to feature-major: xt[pi, s, kt, p] = x[r0+128s+p, 128kt+pi]
            # two calls on the two HWDGE rings (SP + ACT) to overlap
            xt = xt_pool.tile([128, G // 128, KT_F, 128], BF16, name="xt")
            half_s = G // 256
            nc.sync.dma_start(
                xt[:, :half_s, :, :], x_nat[:, :half_s, :], transpose=True
            )
            nc.scalar.dma_start(
                xt[:, half_s:, :, :], x_nat[:, half_s:, :], transpose=True
            )

            # -- h^T = relu(Wf^T x^T + bf):  4 l-tiles of [128, G]
            h_sb = h_pool.tile([128, MT_L, G], BF16, name="h_sb")
            for m in range(MT_L):
                h_ps = psum_mm.tile([128, G], F32, name="h_ps", tag="mm")
                for kt in range(KT_F):
                    for c in range(G // 512):
                        nc.tensor.matmul(
                            h_ps[:, c * 512:(c + 1) * 512],
                            lhsT=wf_sb[:, kt, m * 128:(m + 1) * 128],
                            rhs=xt[:, 4 * c:4 * c + 4, kt, :],
                            start=(kt == 0),
                            stop=(kt == KT_F - 1),
                        )
                # relu + bias, evacuate PSUM -> SBUF bf16 (DVE)
                nc.vector.tensor_scalar(
                    h_sb[:, m, :], h_ps[:],
                    bf_sb[:, m:m + 1], 0.0, AL.add, AL.max,
                )

            # -- a^T = tanh(Wa^T h^T + ba); gt = tanh((Wb^T h^T + bb)/2)
            a_sb = ag_pool.tile([128, MT_D, G], BF16, name="a_sb")
            g_sb = ag_pool.tile([128, MT_D, G], BF16, name="g_sb")
            for m in range(MT_D):
                a_ps = psum_mm.tile([128, G], F32, name="a_ps", tag="mm")
                for kt in range(KT_L):
                    for c in range(G // 512):
                        nc.tensor.matmul(
                            a_ps[:, c * 512:(c + 1) * 512],
                            lhsT=wa_sb[:, kt, m * 128:(m + 1) * 128],
                            rhs=h_sb[:, kt, c * 512:(c + 1) * 512],
                            start=(kt == 0),
                            stop=(kt == KT_L - 1),
                        )
                nc.scalar.activation(
                    a_sb[:, m, :], a_ps[:],
                    mybir.ActivationFunctionType.Tanh,
                    bias=ba_sb[:, m:m + 1],
                )
            for m in range(MT_D):
                g_ps = psum_mm.tile([128, G], F32, name="g_ps", tag="mm")
                for kt in range(KT_L):
                    for c in range(G // 512):
                        nc.tensor.matmul(
                            g_ps[:, c * 512:(c + 1) * 512],
                            lhsT=wb_sb[:, kt, m * 128:(m + 1) * 128],
                            rhs=h_sb[:, kt, c * 512:(c + 1) * 512],
                            start=(kt == 0),
                            stop=(kt == KT_L - 1),
                        )
                # gt = tanh(0.5*z + bb/2); sigmoid(z) = 0.5*gt + 0.5
                nc.scalar.activation(
                    g_sb[:, m, :], g_ps[:],
                    mybir.ActivationFunctionType.Tanh,
                    bias=bbh_sb[:, m:m + 1], scale=0.5,
                )
                nc.vector.tensor_scalar(
                    g_sb[:, m, :], g_sb[:, m, :], 0.5, 0.5, AL.mult, AL.add,
                )
            # ag = a * g (in place into a_sb)
            for m in range(MT_D):
                nc.vector.tensor_tensor(
                    a_sb[:, m, :], a_sb[:, m, :], g_sb[:, m, :], AL.mult,
                )

            # -- s = Wc^T (a*g):  [1, G]
            s_ps = psum_vec.tile([1, G], F32, name="s_ps", tag="vec")
            for kt in range(MT_D):
                for c in range(G // 512):
                    nc.tensor.matmul(
                        s_ps[:, c * 512:(c + 1) * 512],
                        lhsT=wc_sb[:, kt, :],
                        rhs=a_sb[:, kt, c * 512:(c + 1) * 512],
                        start=(kt == 0),
                        stop=(kt == MT_D - 1),
                    )
            s_sb = s_pool.tile([1, G], BF16, name="s_sb")
            nc.vector.tensor_scalar(
                s_sb[:], s_ps[:], bc_sb[0:1, 0:1], None, AL.add,
            )
            if rows < G:
                nc.vector.memset(s_sb[:, rows:], NEG_FILL)

            # -- broadcast s to all partitions, then e = exp(s) with running sum
            sbc_ps = psum_vec.tile([128, G], F32, name="sbc_ps", tag="vec")
            for c in range(G // 512):
                nc.tensor.matmul(
                    sbc_ps[:, c * 512:(c + 1) * 512],
                    lhsT=ones_sb[:],
                    rhs=s_sb[:, c * 512:(c + 1) * 512],
                    start=True, stop=True,
                )
            e_sb = e_pool.tile([128, G], BF16, name="e_sb")
            nc.scalar.activation(
                e_sb[:], sbc_ps[:],
                mybir.ActivationFunctionType.Exp,
                accum_out=S_parts[:, g:g + 1],
            )

            # -- pooled numerator partials: P[m] += sum_r e_r * h^T[m][:, r]
            # (tensor_tensor_reduce is a custom DVE op this runtime lacks,
            # so use plain mult + reduce.)
            for m in range(MT_L):
                nc.vector.tensor_tensor(
                    junk[:], h_sb[:, m, :], e_sb[:], AL.mult,
                )
                nc.vector.tensor_reduce(
                    P_parts[:, m * NGROUPS + g:m * NGROUPS + g + 1],
                    junk[:],
                    axis=mybir.AxisListType.X, op=AL.add,
                )

    # ---- reduce partials, all-reduce across cores, final logits ----
    P_final = fpool.tile([128, MT_L * B], F32)   # col = m*B + b
    S_final = fpool.tile([128, B], F32)
    for b in range(B):
        for m in range(MT_L):
            nc.vector.tensor_reduce(
                P_final[:, m * B + b:m * B + b + 1],
                P_parts[:, m * NGROUPS + b * GB:m * NGROUPS + (b + 1) * GB],
                axis=mybir.AxisListType.X, op=AL.add,
            )
        nc.vector.tensor_reduce(
            S_final[:, b:b + 1],
            S_parts[:, b * GB:(b + 1) * GB],
            axis=mybir.AxisListType.X, op=AL.add,
        )

    if not collective:
        for b in range(B):
            nc.sync.dma_start(
                out_t.ap()[b, 0:L].rearrange("(m p) -> p m", p=128),
                P_final[:, b::B],
            )
            nc.sync.dma_start(out_t.ap()[b, L:L + 1], S_final[0:1, b:b + 1])
        ctx.close()
        return

    cc_in = dram_pool.tile([B, L + 1], F32, name="cc_in")
    cc_out = dram_pool.tile([B, L + 1], F32, name="cc_out", addr_space="Shared")
    for b in range(B):
        nc.sync.dma_start(
            cc_in[b, 0:L].rearrange("(m p) -> p m", p=128),
            P_final[:, b::B],
        )
        nc.sync.dma_start(cc_in[b, L:L + 1], S_final[0:1, b:b + 1])

    nc.gpsimd.collective_compute(
        "AllReduce",
        AL.add,
        replica_groups=[list(range(CORES))],
        ins=[cc_in[:, :]],
        outs=[cc_out[:, :]],
    )

    pooledT = fpool.tile([128, MT_L, B], F32)   # numerators, feature-major
    for b in range(B):
        nc.sync.dma_start(
            pooledT[:, :, b],
            cc_out[b, 0:L].rearrange("(m p) -> p m", p=128),
        )
    S_col = fpool.tile([B, 1], F32)
    nc.sync.dma_start(S_col[:], cc_out[:, L:L + 1])
    S_row = fpool.tile([1, B], F32)
    nc.sync.dma_start(S_row[:], cc_out[:, L:L + 1].rearrange("b o -> o b"))
    S_inv = fpool.tile([B, 1], F32)
    nc.vector.reciprocal(S_inv[:], S_col[:])

    # logits[b, c] = (sum_l P[b, l] Wcls[l, c] + S_b * bcls[c]) / S_b
    lg_ps = psum_vec.tile([B, NCLS], F32, name="lg_ps", tag="vec")
    for kt in range(MT_L):
        nc.tensor.matmul(
            lg_ps[:],
            lhsT=pooledT[:, kt, :],
            rhs=wcls_sb[:, kt, :],
            start=(kt == 0), stop=False,
        )
    nc.tensor.matmul(
        lg_ps[:], lhsT=S_row[:], rhs=bcls_sb[:], start=False, stop=True,
    )
    lg_sb = fpool.tile([B, NCLS], F32)
    nc.vector.tensor_scalar(lg_sb[:], lg_ps[:], S_inv[:], None, AL.mult)
    nc.sync.dma_start(out_t.ap()[:, :], lg_sb[:])
    ctx.close()


_NC_CACHE = None


def _get_nc():
    global _NC_CACHE
    if _NC_CACHE is None:
        _NC_CACHE = build_kernel()
    return _NC_CACHE


def make_in_maps(inputs):
    x = np.asarray(inputs["x"], dtype=np.float32)
    wf = np.asarray(inputs["Wf"], dtype=np.float32).astype(ml_dtypes.bfloat16)
    wa = np.asarray(inputs["Wa"], dtype=np.float32).astype(ml_dtypes.bfloat16)
    wb = np.asarray(inputs["Wb"], dtype=np.float32).astype(ml_dtypes.bfloat16)
    wc = np.asarray(inputs["Wc"], dtype=np.float32).astype(ml_dtypes.bfloat16)
    wcls = np.asarray(inputs["Wcls"], dtype=np.float32)
    bf = np.asarray(inputs["bf"], dtype=np.float32)
    ba = np.asarray(inputs["ba"], dtype=np.float32)
    bbh = np.asarray(inputs["bb"], dtype=np.float32) * 0.5
    bc = np.asarray(inputs["bc"], dtype=np.float32).reshape([1])
    bcls = np.asarray(inputs["bcls"], dtype=np.float32)

    shared = {
        "wf": wf, "wa": wa, "wb": wb, "wc": wc, "wcls": wcls,
        "bf": bf, "ba": ba, "bbh": bbh, "bc": bc, "bcls": bcls,
    }
    in_maps = []
    for c in range(CORES):
        shard = np.ascontiguousarray(x[:, c * NS:(c + 1) * NS, :])
        in_maps.append({"x": shard, **shared})
    return in_maps


def kernel(**inputs) -> np.ndarray:
    nc = _get_nc()
    in_maps = make_in_maps(inputs)
    res = run_bass_kernel_spmd(nc, in_maps, core_ids=list(range(CORES)))
    return np.asarray(res.results[0]["out"], dtype=np.float32)


if __name__ == "__main__":
    nc = build_kernel()
    print("build OK, instructions:", len(nc.m.functions[0].instructions)
          if hasattr(nc.m.functions[0], "instructions") else "?")


# revision 13
# speedup vs baseline: 1.5540x; 1.2895x over previous
"""Trainium2 Bass kernel for nn_AttentionGated (gated-attention MIL pooling).

Math (per batch b):
  h = relu(x @ Wf + bf)            [B, N, L]    L=512
  a = tanh(h @ Wa + ba)            [B, N, D]    D=384
  g = sigmoid(h @ Wb + bb)         [B, N, D]
  s = (a*g) @ Wc + bc              [B, N, 1]
  A = softmax(s over N)            [B, 1, N]
  pooled = A @ h                   [B, 1, L]
  logits = pooled @ Wcls + bcls    [B, 2]

Strategy: shard N across 8 cores (5000 instances/core/batch). Each core
streams its x shard (fp32, cast to bf16 in-flight by SWDGE DMA), DMA-
transposes to feature-major, runs the fused pipeline in bf16 on PE/ACT/DVE,
accumulates unnormalized pooled numerators P_b = sum_n exp(s_n) h_n and
denominators S_b = sum_n exp(s_n) (scores are tightly bounded, so no max
subtraction is needed), then a single AllReduce(add) over [P|S] combines
cores and every core computes the final logits.

sigmoid(x) = 0.5*tanh(x/2) + 0.5 so ACT only needs the exp_and_others
table set (tanh + exp) -> no per-group table switches.
"""

import os
import sys

for _p in ("/opt/trn_rl_repo", "/root/.axon_site/_ro/trn_rl_repo"):
    if os.path.isdir(_p) and _p not in sys.path:
        sys.path.insert(0, _p)

from contextlib import ExitStack

import ml_dtypes
import numpy as np

import concourse.bass as bass
import concourse.mybir as mybir
import concourse.tile as tile
from concourse import bacc
from concourse.bass_utils import run_bass_kernel_spmd

F32 = mybir.dt.float32
BF16 = mybir.dt.bfloat16

B, N, D_IN, L, D, NCLS = 2, 40000, 1024, 512, 384, 2
CORES = 8
NS = N // CORES          # 5000 instances per core per batch
G = 1024                 # rows per processing group
GB = 5                   # groups per batch (4 full + 1 tail)
TAIL = NS - (GB - 1) * G  # 904 real rows in the tail group
NGROUPS = B * GB         # 10
KT_F = D_IN // 128       # 8  k-tiles for Wf
MT_L = L // 128          # 4  l-tiles
KT_L = L // 128          # 4  k-tiles for Wa/Wb
MT_D = D // 128          # 3  d-tiles
NEG_FILL = -40.0         # exp(-40) ~ 4e-18: kills padded rows


def build_kernel(collective=True):
    nc = bacc.Bacc(
        "TRN2",
        target_bir_lowering=False,
        debug=False,
        num_devices=CORES,
    )

    x_t = nc.dram_tensor("x", [B, NS, D_IN], F32, kind="ExternalInput")
    wf_t = nc.dram_tensor("wf", [D_IN, L], BF16, kind="ExternalInput")
    wa_t = nc.dram_tensor("wa", [L, D], BF16, kind="ExternalInput")
    wb_t = nc.dram_tensor("wb", [L, D], BF16, kind="ExternalInput")
    wc_t = nc.dram_tensor("wc", [D, 1], BF16, kind="ExternalInput")
    wcls_t = nc.dram_tensor("wcls", [L, NCLS], F32, kind="ExternalInput")
    bf_t = nc.dram_tensor("bf", [L], F32, kind="ExternalInput")
    ba_t = nc.dram_tensor("ba", [D], F32, kind="ExternalInput")
    bbh_t = nc.dram_tensor("bbh", [D], F32, kind="ExternalInput")  # bb/2
    bc_t = nc.dram_tensor("bc", [1], F32, kind="ExternalInput")
    bcls_t = nc.dram_tensor("bcls", [NCLS], F32, kind="ExternalInput")
    if collective:
        out_t = nc.dram_tensor("out", [B, NCLS], F32, kind="ExternalOutput")
    else:
        out_t = nc.dram_tensor("out_ps", [B, L + 1], F32, kind="ExternalOutput")

    with tile.TileContext(nc) as tc:
        _body(tc, nc, x_t, wf_t, wa_t, wb_t, wc_t, wcls_t,
              bf_t, ba_t, bbh_t, bc_t, bcls_t, out_t, collective)

    nc.compile()
    return nc


def _body(tc, nc, x_t, wf_t, wa_t, wb_t, wc_t, wcls_t,
          bf_t, ba_t, bbh_t, bc_t, bcls_t, out_t, collective=True):
    AL = mybir.AluOpType

    ctx = ExitStack()
    wpool = ctx.enter_context(tc.tile_pool(name="weights", bufs=1))
    fpool = ctx.enter_context(tc.tile_pool(name="final", bufs=1))
    stage_pool = ctx.enter_context(tc.tile_pool(name="stage", bufs=3))
    xnat_pool = ctx.enter_context(tc.tile_pool(name="xnat", bufs=2))
    xt_pool = ctx.enter_context(tc.tile_pool(name="xt", bufs=2))
    h_pool = ctx.enter_context(tc.tile_pool(name="h", bufs=2))
    ag_pool = ctx.enter_context(tc.tile_pool(name="ag", bufs=2))
    e_pool = ctx.enter_context(tc.tile_pool(name="e", bufs=2))
    s_pool = ctx.enter_context(tc.tile_pool(name="s", bufs=2))
    psum_mm = ctx.enter_context(tc.tile_pool(name="psum_mm", bufs=3, space="PSUM"))
    psum_vec = ctx.enter_context(tc.tile_pool(name="psum_vec", bufs=1, space="PSUM"))
    dram_pool = ctx.enter_context(tc.tile_pool(name="dram", bufs=1, space="DRAM"))

    # ---- weights / constants (loaded once) ----
    wf_sb = wpool.tile([128, KT_F, L], BF16)
    nc.sync.dma_start(wf_sb[:], wf_t.ap().rearrange("(kt p) l -> p kt l", p=128))
    wa_sb = wpool.tile([128, KT_L, D], BF16)
    nc.sync.dma_start(wa_sb[:], wa_t.ap().rearrange("(kt p) d -> p kt d", p=128))
    wb_sb = wpool.tile([128, KT_L, D], BF16)
    nc.sync.dma_start(wb_sb[:], wb_t.ap().rearrange("(kt p) d -> p kt d", p=128))
    wc_sb = wpool.tile([128, MT_D, 1], BF16)
    nc.sync.dma_start(wc_sb[:], wc_t.ap().rearrange("(kt p) o -> p kt o", p=128))
    wcls_sb = wpool.tile([128, MT_L, NCLS], F32)
    nc.sync.dma_start(wcls_sb[:], wcls_t.ap().rearrange("(kt p) c -> p kt c", p=128))

    bf_sb = wpool.tile([128, MT_L], F32)
    nc.sync.dma_start(bf_sb[:], bf_t.ap().rearrange("(m p) -> p m", p=128))
    ba_sb = wpool.tile([128, MT_D], F32)
    nc.sync.dma_start(ba_sb[:], ba_t.ap().rearrange("(m p) -> p m", p=128))
    bbh_sb = wpool.tile([128, MT_D], F32)
    nc.sync.dma_start(bbh_sb[:], bbh_t.ap().rearrange("(m p) -> p m", p=128))
    bc_sb = wpool.tile([1, 1], F32)
    nc.sync.dma_start(bc_sb[:], bc_t.ap()[None, :])
    bcls_sb = wpool.tile([1, NCLS], F32)
    nc.sync.dma_start(bcls_sb[:], bcls_t.ap()[None, :])

    ones_sb = wpool.tile([1, 128], BF16)
    nc.vector.memset(ones_sb[:], 1.0)

    # persistent accumulators
    P_parts = fpool.tile([128, MT_L * NGROUPS], F32)   # col = m*NGROUPS + g
    S_parts = fpool.tile([128, NGROUPS], F32)
    junk = fpool.tile([128, G], BF16)                  # TTR product sink

    # ---- main pipeline over groups ----
    for b in range(B):
        for gi in range(GB):
            g = b * GB + gi
            r0 = gi * G
            rows = G if gi < GB - 1 else TAIL

            # -- load x rows fp32 (HWDGE), cast fp32->bf16 on idle GPSIMD --
            # (SWDGE dtype-cast DMA crashes NRT on this runtime, so the
            # cast runs as a gpsimd tensor_copy instead.)
            HS = G // 256                            # 4 subtiles per half
            x_nat = xnat_pool.tile([128, G // 128, D_IN], BF16, name="x_nat")
            for hf in range(2):
                st = stage_pool.tile([128, HS, D_IN], F32, name="st")
                hr0 = r0 + hf * HS * 128             # first row of this half
                hrows = min(rows - hf * HS * 128, HS * 128)
                if hrows == HS * 128:
                    nc.sync.dma_start(
                        st[:],
                        x_t.ap()[b, hr0:hr0 + HS * 128, :]
                        .rearrange("(s p) d -> p s d", p=128),
                    )
                else:
                    full_s = hrows // 128            # full subtiles in half
                    rem = hrows - full_s * 128       # leftover rows
                    nc.vector.memset(st[:, full_s:, :], 0.0)
                    if full_s:
                        nc.sync.dma_start(
                            st[:, :full_s, :],
                            x_t.ap()[b, hr0:hr0 + full_s * 128, :]
                            .rearrange("(s p) d -> p s d", p=128),
                        )
                    if rem:
                        nc.sync.dma_start(
                            st[:rem, full_s, :],
                            x_t.ap()[b, hr0 + full_s * 128:hr0 + hrows, :],
                        )
                # split the cast between ACT (own SBUF ports) and DVE
                # (2x two-port mode) -- gpsimd's software copy is ~4x
                # slower and its SBUF port use stalls concurrent DVE ops.
                if hf == 0:
                    nc.scalar.copy(x_nat[:, hf * HS:(hf + 1) * HS, :], st[:])
                else:
                    nc.vector.tensor_copy(x_nat[:, hf * HS:(hf + 1) * HS, :], st[:])

            # -- transpose to feature-major: xt[pi, s, kt, p] = x[r0+128s+p, 128kt+pi]
            xt = xt_pool.tile([128, G // 128, KT_F, 128], BF16, name="xt")
            nc.sync.dma_start(xt[:], x_nat[:], transpose=True)

            # -- h^T = relu(Wf^T x^T + bf):  4 l-tiles of [128, G]
            h_sb = h_pool.tile([128, MT_L, G], BF16, name="h_sb")
            for m in range(MT_L):
                h_ps = psum_mm.tile([128, G], F32, name="h_ps", tag="mm")
                for kt in range(KT_F):
                    for c in range(G // 512):
                        nc.tensor.matmul(
                            h_ps[:, c * 512:(c + 1) * 512],
                            lhsT=wf_sb[:, kt, m * 128:(m + 1) * 128],
                            rhs=xt[:, 4 * c:4 * c + 4, kt, :],
                            start=(kt == 0),
                            stop=(kt == KT_F - 1),
                        )
                # relu + bias, evacuate PSUM -> SBUF bf16 (DVE)
                nc.vector.tensor_scalar(
                    h_sb[:, m, :], h_ps[:],
                    bf_sb[:, m:m + 1], 0.0, AL.add, AL.max,
                )

            # -- a^T = tanh(Wa^T h^T + ba); gt = tanh((Wb^T h^T + bb)/2)
            a_sb = ag_pool.tile([128, MT_D, G], BF16, name="a_sb")
            g_sb = ag_pool.tile([128, MT_D, G], BF16, name="g_sb")
            for m in range(MT_D):
                a_ps = psum_mm.tile([128, G], F32, name="a_ps", tag="mm")
                for kt in range(KT_L):
                    for c in range(G // 512):
                        nc.tensor.matmul(
                            a_ps[:, c * 512:(c + 1) * 512],
                            lhsT=wa_sb[:, kt, m * 128:(m + 1) * 128],
                            rhs=h_sb[:, kt, c * 512:(c + 1) * 512],
                            start=(kt == 0),
                            stop=(kt == KT_L - 1),
                        )
                nc.scalar.activation(
                    a_sb[:, m, :], a_ps[:],
                    mybir.ActivationFunctionType.Tanh,
                    bias=ba_sb[:, m:m + 1],
                )
            for m in range(MT_D):
                g_ps = psum_mm.tile([128, G], F32, name="g_ps", tag="mm")
                for kt in range(KT_L):
                    for c in range(G // 512):
                        nc.tensor.matmul(
                            g_ps[:, c * 512:(c + 1) * 512],
                            lhsT=wb_sb[:, kt, m * 128:(m + 1) * 128],
                            rhs=h_sb[:, kt, c * 512:(c + 1) * 512],
                            start=(kt == 0),
                            stop=(kt == KT_L - 1),
                        )
                # gt = tanh(0.5*z + bb/2); sigmoid(z) = 0.5*gt + 0.5
                nc.scalar.activation(
                    g_sb[:, m, :], g_ps[:],
                    mybir.ActivationFunctionType.Tanh,
                    bias=bbh_sb[:, m:m + 1], scale=0.5,
                )
                nc.vector.tensor_scalar(
                    g_sb[:, m, :], g_sb[:, m, :], 0.5, 0.5, AL.mult, AL.add,
                )
            # ag = a * g (in place into a_sb)
            for m in range(MT_D):
                nc.vector.tensor_tensor(
                    a_sb[:, m, :], a_sb[:, m, :], g_sb[:, m, :], AL.mult,
                )

            # -- s = Wc^T (a*g):  [1, G]
            s_ps = psum_vec.tile([1, G], F32, name="s_ps", tag="vec")
            for kt in range(MT_D):
                for c in range(G // 512):
                    nc.tensor.matmul(
                        s_ps[:, c * 512:(c + 1) * 512],
                        lhsT=wc_sb[:, kt, :],
                        rhs=a_sb[:, kt, c * 512:(c + 1) * 512],
                        start=(kt == 0),
                        stop=(kt == MT_D - 1),
                    )
            s_sb = s_pool.tile([1, G], BF16, name="s_sb")
            nc.vector.tensor_scalar(
                s_sb[:], s_ps[:], bc_sb[0:1, 0:1], None, AL.add,
            )
            if rows < G:
                nc.vector.memset(s_sb[:, rows:], NEG_FILL)

            # -- broadcast s to all partitions, then e = exp(s) with running sum
            sbc_ps = psum_vec.tile([128, G], F32, name="sbc_ps", tag="vec")
            for c in range(G // 512):
                nc.tensor.matmul(
                    sbc_ps[:, c * 512:(c + 1) * 512],
                    lhsT=ones_sb[:],
                    rhs=s_sb[:, c * 512:(c + 1) * 512],
                    start=True, stop=True,
                )
            e_sb = e_pool.tile([128, G], BF16, name="e_sb")
            nc.scalar.activation(
                e_sb[:], sbc_ps[:],
                mybir.ActivationFunctionType.Exp,
                accum_out=S_parts[:, g:g + 1],
            )

            # -- pooled numerator partials: P[m] += sum_r e_r * h^T[m][:, r]
            # (tensor_tensor_reduce is a custom DVE op this runtime lacks,
            # so use plain mult + reduce.)
            for m in range(MT_L):
                nc.vector.tensor_tensor(
                    junk[:], h_sb[:, m, :], e_sb[:], AL.mult,
                )
                nc.vector.tensor_reduce(
                    P_parts[:, m * NGROUPS + g:m * NGROUPS + g + 1],
                    junk[:],
                    axis=mybir.AxisListType.X, op=AL.add,
                )

    # ---- reduce partials, all-reduce across cores, final logits ----
    P_final = fpool.tile([128, MT_L * B], F32)   # col = m*B + b
    S_final = fpool.tile([128, B], F32)
    for b in range(B):
        for m in range(MT_L):
            nc.vector.tensor_reduce(
                P_final[:, m * B + b:m * B + b + 1],
                P_parts[:, m * NGROUPS + b * GB:m * NGROUPS + (b + 1) * GB],
                axis=mybir.AxisListType.X, op=AL.add,
            )
        nc.vector.tensor_reduce(
            S_final[:, b:b + 1],
            S_parts[:, b * GB:(b + 1) * GB],
            axis=mybir.AxisListType.X, op=AL.add,
        )

    if not collective:
        for b in range(B):
            nc.sync.dma_start(
                out_t.ap()[b, 0:L].rearrange("(m p) -> p m", p=128),
                P_final[:, b::B],
            )
            nc.sync.dma_start(out_t.ap()[b, L:L + 1], S_final[0:1, b:b + 1])
        ctx.close()
        return

    cc_in = dram_pool.tile([B, L + 1], F32, name="cc_in")
    cc_out = dram_pool.tile([B, L + 1], F32, name="cc_out", addr_space="Shared")
    for b in range(B):
        nc.sync.dma_start(
            cc_in[b, 0:L].rearrange("(m p) -> p m", p=128),
            P_final[:, b::B],
        )
        nc.sync.dma_start(cc_in[b, L:L + 1], S_final[0:1, b:b + 1])

    nc.gpsimd.collective_compute(
        "AllReduce",
        AL.add,
        replica_groups=[list(range(CORES))],
        ins=[cc_in[:, :]],
        outs=[cc_out[:, :]],
    )

    pooledT = fpool.tile([128, MT_L, B], F32)   # numerators, feature-major
    for b in range(B):
        nc.sync.dma_start(
            pooledT[:, :, b],
            cc_out[b, 0:L].rearrange("(m p) -> p m", p=128),
        )
    S_col = fpool.tile([B, 1], F32)
    nc.sync.dma_start(S_col[:], cc_out[:, L:L + 1])
    S_row = fpool.tile([1, B], F32)
    nc.sync.dma_start(S_row[:], cc_out[:, L:L + 1].rearrange("b o -> o b"))
    S_inv = fpool.tile([B, 1], F32)
    nc.vector.reciprocal(S_inv[:], S_col[:])

    # logits[b, c] = (sum_l P[b, l] Wcls[l, c] + S_b * bcls[c]) / S_b
    lg_ps = psum_vec.tile([B, NCLS], F32, name="lg_ps", tag="vec")
    for kt in range(MT_L):
        nc.tensor.matmul(
            lg_ps[:],
            lhsT=pooledT[:, kt, :],
            rhs=wcls_sb[:, kt, :],
            start=(kt == 0), stop=False,
        )
    nc.tensor.matmul(
        lg_ps[:], lhsT=S_row[:], rhs=bcls_sb[:], start=False, stop=True,
    )
    lg_sb = fpool.tile([B, NCLS], F32)
    nc.vector.tensor_scalar(lg_sb[:], lg_ps[:], S_inv[:], None, AL.mult)
    nc.sync.dma_start(out_t.ap()[:, :], lg_sb[:])
    ctx.close()


_NC_CACHE = None


def _get_nc():
    global _NC_CACHE
    if _NC_CACHE is None:
        _NC_CACHE = build_kernel()
    return _NC_CACHE


def make_in_maps(inputs):
    x = np.asarray(inputs["x"], dtype=np.float32)
    wf = np.asarray(inputs["Wf"], dtype=np.float32).astype(ml_dtypes.bfloat16)
    wa = np.asarray(inputs["Wa"], dtype=np.float32).astype(ml_dtypes.bfloat16)
    wb = np.asarray(inputs["Wb"], dtype=np.float32).astype(ml_dtypes.bfloat16)
    wc = np.asarray(inputs["Wc"], dtype=np.float32).astype(ml_dtypes.bfloat16)
    wcls = np.asarray(inputs["Wcls"], dtype=np.float32)
    bf = np.asarray(inputs["bf"], dtype=np.float32)
    ba = np.asarray(inputs["ba"], dtype=np.float32)
    bbh = np.asarray(inputs["bb"], dtype=np.float32) * 0.5
    bc = np.asarray(inputs["bc"], dtype=np.float32).reshape([1])
    bcls = np.asarray(inputs["bcls"], dtype=np.float32)

    shared = {
        "wf": wf, "wa": wa, "wb": wb, "wc": wc, "wcls": wcls,
        "bf": bf, "ba": ba, "bbh": bbh, "bc": bc, "bcls": bcls,
    }
    in_maps = []
    for c in range(CORES):
        shard = np.ascontiguousarray(x[:, c * NS:(c + 1) * NS, :])
        in_maps.append({"x": shard, **shared})
    return in_maps


def kernel(**inputs) -> np.ndarray:
    nc = _get_nc()
    in_maps = make_in_maps(inputs)
    res = run_bass_kernel_spmd(nc, in_maps, core_ids=list(range(CORES)))
    return np.asarray(res.results[0]["out"], dtype=np.float32)


if __name__ == "__main__":
    nc = build_kernel()
    print("build OK, instructions:", len(nc.m.functions[0].instructions)
          if hasattr(nc.m.functions[0], "instructions") else "?")


# revision 14
# speedup vs baseline: 1.5912x; 1.0239x over previous
"""Trainium2 Bass kernel for nn_AttentionGated (gated-attention MIL pooling).

Math (per batch b):
  h = relu(x @ Wf + bf)            [B, N, L]    L=512
  a = tanh(h @ Wa + ba)            [B, N, D]    D=384
  g = sigmoid(h @ Wb + bb)         [B, N, D]
  s = (a*g) @ Wc + bc              [B, N, 1]
  A = softmax(s over N)            [B, 1, N]
  pooled = A @ h                   [B, 1, L]
  logits = pooled @ Wcls + bcls    [B, 2]

Strategy: shard N across 8 cores (5000 instances/core/batch). Each core
streams its x shard (fp32, cast to bf16 in-flight by SWDGE DMA), DMA-
transposes to feature-major, runs the fused pipeline in bf16 on PE/ACT/DVE,
accumulates unnormalized pooled numerators P_b = sum_n exp(s_n) h_n and
denominators S_b = sum_n exp(s_n) (scores are tightly bounded, so no max
subtraction is needed), then a single AllReduce(add) over [P|S] combines
cores and every core computes the final logits.

sigmoid(x) = 0.5*tanh(x/2) + 0.5 so ACT only needs the exp_and_others
table set (tanh + exp) -> no per-group table switches.
"""

import os
import sys

for _p in ("/opt/trn_rl_repo", "/root/.axon_site/_ro/trn_rl_repo"):
    if os.path.isdir(_p) and _p not in sys.path:
        sys.path.insert(0, _p)

from contextlib import ExitStack

import ml_dtypes
import numpy as np

import concourse.bass as bass
import concourse.mybir as mybir
import concourse.tile as tile
from concourse import bacc
from concourse.bass_utils import run_bass_kernel_spmd

F32 = mybir.dt.float32
BF16 = mybir.dt.bfloat16
FP8 = mybir.dt.float8e4

B, N, D_IN, L, D, NCLS = 2, 40000, 1024, 512, 384, 2
CORES = 8
NS = N // CORES          # 5000 instances per core per batch
G = 1024                 # rows per processing group
GB = 5                   # groups per batch (4 full + 1 tail)
TAIL = NS - (GB - 1) * G  # 904 real rows in the tail group
NGROUPS = B * GB         # 10
KT_F = D_IN // 128       # 8  k-tiles for Wf
MT_L = L // 128          # 4  l-tiles
KT_L = L // 128          # 4  k-tiles for Wa/Wb
MT_D = D // 128          # 3  d-tiles
NEG_FILL = -40.0         # exp(-40) ~ 4e-18: kills padded rows


def build_kernel(collective=True):
    nc = bacc.Bacc(
        "TRN2",
        target_bir_lowering=False,
        debug=False,
        num_devices=CORES,
    )

    x_t = nc.dram_tensor("x", [B, NS, D_IN], F32, kind="ExternalInput")
    wf_t = nc.dram_tensor("wf", [D_IN, L], BF16, kind="ExternalInput")
    wa_t = nc.dram_tensor("wa", [L, D], FP8, kind="ExternalInput")
    wb_t = nc.dram_tensor("wb", [L, D], FP8, kind="ExternalInput")
    wc_t = nc.dram_tensor("wc", [D, 1], BF16, kind="ExternalInput")
    wcls_t = nc.dram_tensor("wcls", [L, NCLS], F32, kind="ExternalInput")
    bf_t = nc.dram_tensor("bf", [L], F32, kind="ExternalInput")
    ba_t = nc.dram_tensor("ba", [D], F32, kind="ExternalInput")
    bbh_t = nc.dram_tensor("bbh", [D], F32, kind="ExternalInput")  # bb/2
    bc_t = nc.dram_tensor("bc", [1], F32, kind="ExternalInput")
    bcls_t = nc.dram_tensor("bcls", [NCLS], F32, kind="ExternalInput")
    if collective:
        out_t = nc.dram_tensor("out", [B, NCLS], F32, kind="ExternalOutput")
    else:
        out_t = nc.dram_tensor("out_ps", [B, L + 1], F32, kind="ExternalOutput")

    with tile.TileContext(nc) as tc:
        _body(tc, nc, x_t, wf_t, wa_t, wb_t, wc_t, wcls_t,
              bf_t, ba_t, bbh_t, bc_t, bcls_t, out_t, collective)

    nc.compile()
    return nc


def _body(tc, nc, x_t, wf_t, wa_t, wb_t, wc_t, wcls_t,
          bf_t, ba_t, bbh_t, bc_t, bcls_t, out_t, collective=True):
    AL = mybir.AluOpType

    ctx = ExitStack()
    wpool = ctx.enter_context(tc.tile_pool(name="weights", bufs=1))
    fpool = ctx.enter_context(tc.tile_pool(name="final", bufs=1))
    stage_pool = ctx.enter_context(tc.tile_pool(name="stage", bufs=3))
    xnat_pool = ctx.enter_context(tc.tile_pool(name="xnat", bufs=2))
    xt_pool = ctx.enter_context(tc.tile_pool(name="xt", bufs=2))
    h_pool = ctx.enter_context(tc.tile_pool(name="h", bufs=2))
    ag_pool = ctx.enter_context(tc.tile_pool(name="ag", bufs=2))
    e_pool = ctx.enter_context(tc.tile_pool(name="e", bufs=2))
    s_pool = ctx.enter_context(tc.tile_pool(name="s", bufs=2))
    psum_mm = ctx.enter_context(tc.tile_pool(name="psum_mm", bufs=3, space="PSUM"))
    psum_vec = ctx.enter_context(tc.tile_pool(name="psum_vec", bufs=1, space="PSUM"))
    dram_pool = ctx.enter_context(tc.tile_pool(name="dram", bufs=1, space="DRAM"))

    # ---- weights / constants (loaded once) ----
    wf_sb = wpool.tile([128, KT_F, L], BF16)
    nc.sync.dma_start(wf_sb[:], wf_t.ap().rearrange("(kt p) l -> p kt l", p=128))
    wa_sb = wpool.tile([128, KT_L, D], FP8)
    nc.sync.dma_start(wa_sb[:], wa_t.ap().rearrange("(kt p) d -> p kt d", p=128))
    wb_sb = wpool.tile([128, KT_L, D], FP8)
    nc.sync.dma_start(wb_sb[:], wb_t.ap().rearrange("(kt p) d -> p kt d", p=128))
    wc_sb = wpool.tile([128, MT_D, 1], BF16)
    nc.sync.dma_start(wc_sb[:], wc_t.ap().rearrange("(kt p) o -> p kt o", p=128))
    wcls_sb = wpool.tile([128, MT_L, NCLS], F32)
    nc.sync.dma_start(wcls_sb[:], wcls_t.ap().rearrange("(kt p) c -> p kt c", p=128))

    bf_sb = wpool.tile([128, MT_L], F32)
    nc.sync.dma_start(bf_sb[:], bf_t.ap().rearrange("(m p) -> p m", p=128))
    ba_sb = wpool.tile([128, MT_D], F32)
    nc.sync.dma_start(ba_sb[:], ba_t.ap().rearrange("(m p) -> p m", p=128))
    bbh_sb = wpool.tile([128, MT_D], F32)
    nc.sync.dma_start(bbh_sb[:], bbh_t.ap().rearrange("(m p) -> p m", p=128))
    bc_sb = wpool.tile([1, 1], F32)
    nc.sync.dma_start(bc_sb[:], bc_t.ap()[None, :])
    bcls_sb = wpool.tile([1, NCLS], F32)
    nc.sync.dma_start(bcls_sb[:], bcls_t.ap()[None, :])

    ones_sb = wpool.tile([1, 128], BF16)
    nc.vector.memset(ones_sb[:], 1.0)

    # persistent accumulators
    P_parts = fpool.tile([128, MT_L * NGROUPS], F32)   # col = m*NGROUPS + g
    S_parts = fpool.tile([128, NGROUPS], F32)
    junk = fpool.tile([128, G], BF16)                  # TTR product sink

    # ---- main pipeline over groups ----
    for b in range(B):
        for gi in range(GB):
            g = b * GB + gi
            r0 = gi * G
            rows = G if gi < GB - 1 else TAIL

            # -- load x rows fp32 (HWDGE), cast fp32->bf16 on idle GPSIMD --
            # (SWDGE dtype-cast DMA crashes NRT on this runtime, so the
            # cast runs as a gpsimd tensor_copy instead.)
            HS = G // 256                            # 4 subtiles per half
            x_nat = xnat_pool.tile([128, G // 128, D_IN], BF16, name="x_nat")
            for hf in range(2):
                st = stage_pool.tile([128, HS, D_IN], F32, name="st")
                hr0 = r0 + hf * HS * 128             # first row of this half
                hrows = min(rows - hf * HS * 128, HS * 128)
                if hrows == HS * 128:
                    nc.sync.dma_start(
                        st[:],
                        x_t.ap()[b, hr0:hr0 + HS * 128, :]
                        .rearrange("(s p) d -> p s d", p=128),
                    )
                else:
                    full_s = hrows // 128            # full subtiles in half
                    rem = hrows - full_s * 128       # leftover rows
                    nc.vector.memset(st[:, full_s:, :], 0.0)
                    if full_s:
                        nc.sync.dma_start(
                            st[:, :full_s, :],
                            x_t.ap()[b, hr0:hr0 + full_s * 128, :]
                            .rearrange("(s p) d -> p s d", p=128),
                        )
                    if rem:
                        nc.sync.dma_start(
                            st[:rem, full_s, :],
                            x_t.ap()[b, hr0 + full_s * 128:hr0 + hrows, :],
                        )
                # split the cast between ACT (own SBUF ports) and DVE
                # (2x two-port mode) -- gpsimd's software copy is ~4x
                # slower and its SBUF port use stalls concurrent DVE ops.
                if hf == 0:
                    nc.scalar.copy(x_nat[:, hf * HS:(hf + 1) * HS, :], st[:])
                else:
                    nc.vector.tensor_copy(x_nat[:, hf * HS:(hf + 1) * HS, :], st[:])

            # -- transpose to feature-major: xt[pi, s, kt, p] = x[r0+128s+p, 128kt+pi]
            xt = xt_pool.tile([128, G // 128, KT_F, 128], BF16, name="xt")
            nc.sync.dma_start(xt[:], x_nat[:], transpose=True)

            # -- h^T = relu(Wf^T x^T + bf):  4 l-tiles of [128, G]
            h_sb = h_pool.tile([128, MT_L, G], FP8, name="h_sb")
            for m in range(MT_L):
                h_ps = psum_mm.tile([128, G], F32, name="h_ps", tag="mm")
                for kt in range(KT_F):
                    for c in range(G // 512):
                        nc.tensor.matmul(
                            h_ps[:, c * 512:(c + 1) * 512],
                            lhsT=wf_sb[:, kt, m * 128:(m + 1) * 128],
                            rhs=xt[:, 4 * c:4 * c + 4, kt, :],
                            start=(kt == 0),
                            stop=(kt == KT_F - 1),
                        )
                # relu + bias, evacuate PSUM -> SBUF bf16 (DVE)
                nc.vector.tensor_scalar(
                    h_sb[:, m, :], h_ps[:],
                    bf_sb[:, m:m + 1], 0.0, AL.add, AL.max,
                )

            # -- a^T = tanh(Wa^T h^T + ba); gt = tanh((Wb^T h^T + bb)/2)
            a_sb = ag_pool.tile([128, MT_D, G], BF16, name="a_sb")
            g_sb = ag_pool.tile([128, MT_D, G], BF16, name="g_sb")
            for m in range(MT_D):
                a_ps = psum_mm.tile([128, G], F32, name="a_ps", tag="mm")
                for tp in range(KT_L // 2):
                    for c in range(G // 512):
                        nc.tensor.matmul(
                            a_ps[:, c * 512:(c + 1) * 512],
                            lhsT=wa_sb[:, 2 * tp:2 * tp + 2, m * 128:(m + 1) * 128],
                            rhs=h_sb[:, 2 * tp:2 * tp + 2, c * 512:(c + 1) * 512],
                            start=(tp == 0),
                            stop=(tp == KT_L // 2 - 1),
                            perf_mode=mybir.MatmulPerfMode.DoubleRow,
                        )
                nc.scalar.activation(
                    a_sb[:, m, :], a_ps[:],
                    mybir.ActivationFunctionType.Tanh,
                    bias=ba_sb[:, m:m + 1],
                )
            for m in range(MT_D):
                g_ps = psum_mm.tile([128, G], F32, name="g_ps", tag="mm")
                for tp in range(KT_L // 2):
                    for c in range(G // 512):
                        nc.tensor.matmul(
                            g_ps[:, c * 512:(c + 1) * 512],
                            lhsT=wb_sb[:, 2 * tp:2 * tp + 2, m * 128:(m + 1) * 128],
                            rhs=h_sb[:, 2 * tp:2 * tp + 2, c * 512:(c + 1) * 512],
                            start=(tp == 0),
                            stop=(tp == KT_L // 2 - 1),
                            perf_mode=mybir.MatmulPerfMode.DoubleRow,
                        )
                # gt = tanh(0.5*z + bb/2); sigmoid(z) = 0.5*gt + 0.5
                nc.scalar.activation(
                    g_sb[:, m, :], g_ps[:],
                    mybir.ActivationFunctionType.Tanh,
                    bias=bbh_sb[:, m:m + 1], scale=0.5,
                )
                nc.vector.tensor_scalar(
                    g_sb[:, m, :], g_sb[:, m, :], 0.5, 0.5, AL.mult, AL.add,
                )
            # ag = a * g (in place into a_sb)
            for m in range(MT_D):
                nc.vector.tensor_tensor(
                    a_sb[:, m, :], a_sb[:, m, :], g_sb[:, m, :], AL.mult,
                )

            # -- s = Wc^T (a*g):  [1, G]
            s_ps = psum_vec.tile([1, G], F32, name="s_ps", tag="vec")
            for kt in range(MT_D):
                for c in range(G // 512):
                    nc.tensor.matmul(
                        s_ps[:, c * 512:(c + 1) * 512],
                        lhsT=wc_sb[:, kt, :],
                        rhs=a_sb[:, kt, c * 512:(c + 1) * 512],
                        start=(kt == 0),
                        stop=(kt == MT_D - 1),
                    )
            s_sb = s_pool.tile([1, G], BF16, name="s_sb")
            nc.vector.tensor_scalar(
                s_sb[:], s_ps[:], bc_sb[0:1, 0:1], None, AL.add,
            )
            if rows < G:
                nc.vector.memset(s_sb[:, rows:], NEG_FILL)

            # -- broadcast s to all partitions, then e = exp(s) with running sum
            sbc_ps = psum_vec.tile([128, G], F32, name="sbc_ps", tag="vec")
            for c in range(G // 512):
                nc.tensor.matmul(
                    sbc_ps[:, c * 512:(c + 1) * 512],
                    lhsT=ones_sb[:],
                    rhs=s_sb[:, c * 512:(c + 1) * 512],
                    start=True, stop=True,
                )
            e_sb = e_pool.tile([128, G], BF16, name="e_sb")
            nc.scalar.activation(
                e_sb[:], sbc_ps[:],
                mybir.ActivationFunctionType.Exp,
                accum_out=S_parts[:, g:g + 1],
            )

            # -- pooled numerator partials: P[m] += sum_r e_r * h^T[m][:, r]
            # (tensor_tensor_reduce is a custom DVE op this runtime lacks,
            # so use plain mult + reduce.)
            for m in range(MT_L):
                nc.vector.tensor_tensor(
                    junk[:], h_sb[:, m, :], e_sb[:], AL.mult,
                )
                nc.vector.tensor_reduce(
                    P_parts[:, m * NGROUPS + g:m * NGROUPS + g + 1],
                    junk[:],
                    axis=mybir.AxisListType.X, op=AL.add,
                )

    # ---- reduce partials, all-reduce across cores, final logits ----
    P_final = fpool.tile([128, MT_L * B], F32)   # col = m*B + b
    S_final = fpool.tile([128, B], F32)
    for b in range(B):
        for m in range(MT_L):
            nc.vector.tensor_reduce(
                P_final[:, m * B + b:m * B + b + 1],
                P_parts[:, m * NGROUPS + b * GB:m * NGROUPS + (b + 1) * GB],
                axis=mybir.AxisListType.X, op=AL.add,
            )
        nc.vector.tensor_reduce(
            S_final[:, b:b + 1],
            S_parts[:, b * GB:(b + 1) * GB],
            axis=mybir.AxisListType.X, op=AL.add,
        )

    if not collective:
        for b in range(B):
            nc.sync.dma_start(
                out_t.ap()[b, 0:L].rearrange("(m p) -> p m", p=128),
                P_final[:, b::B],
            )
            nc.sync.dma_start(out_t.ap()[b, L:L + 1], S_final[0:1, b:b + 1])
        ctx.close()
        return

    cc_in = dram_pool.tile([B, L + 1], F32, name="cc_in")
    cc_out = dram_pool.tile([B, L + 1], F32, name="cc_out", addr_space="Shared")
    for b in range(B):
        nc.sync.dma_start(
            cc_in[b, 0:L].rearrange("(m p) -> p m", p=128),
            P_final[:, b::B],
        )
        nc.sync.dma_start(cc_in[b, L:L + 1], S_final[0:1, b:b + 1])

    nc.gpsimd.collective_compute(
        "AllReduce",
        AL.add,
        replica_groups=[list(range(CORES))],
        ins=[cc_in[:, :]],
        outs=[cc_out[:, :]],
    )

    pooledT = fpool.tile([128, MT_L, B], F32)   # numerators, feature-major
    for b in range(B):
        nc.sync.dma_start(
            pooledT[:, :, b],
            cc_out[b, 0:L].rearrange("(m p) -> p m", p=128),
        )
    S_col = fpool.tile([B, 1], F32)
    nc.sync.dma_start(S_col[:], cc_out[:, L:L + 1])
    S_row = fpool.tile([1, B], F32)
    nc.sync.dma_start(S_row[:], cc_out[:, L:L + 1].rearrange("b o -> o b"))
    S_inv = fpool.tile([B, 1], F32)
    nc.vector.reciprocal(S_inv[:], S_col[:])

    # logits[b, c] = (sum_l P[b, l] Wcls[l, c] + S_b * bcls[c]) / S_b
    lg_ps = psum_vec.tile([B, NCLS], F32, name="lg_ps", tag="vec")
    for kt in range(MT_L):
        nc.tensor.matmul(
            lg_ps[:],
            lhsT=pooledT[:, kt, :],
            rhs=wcls_sb[:, kt, :],
            start=(kt == 0), stop=False,
        )
    nc.tensor.matmul(
        lg_ps[:], lhsT=S_row[:], rhs=bcls_sb[:], start=False, stop=True,
    )
    lg_sb = fpool.tile([B, NCLS], F32)
    nc.vector.tensor_scalar(lg_sb[:], lg_ps[:], S_inv[:], None, AL.mult)
    nc.sync.dma_start(out_t.ap()[:, :], lg_sb[:])
    ctx.close()


_NC_CACHE = None


def _get_nc():
    global _NC_CACHE
    if _NC_CACHE is None:
        _NC_CACHE = build_kernel()
    return _NC_CACHE


def make_in_maps(inputs):
    x = np.asarray(inputs["x"], dtype=np.float32)
    wf = np.asarray(inputs["Wf"], dtype=np.float32).astype(ml_dtypes.bfloat16)
    np_fp8 = mybir.dt.np(FP8)
    wa = np.asarray(inputs["Wa"], dtype=np.float32).astype(np_fp8)
    wb = np.asarray(inputs["Wb"], dtype=np.float32).astype(np_fp8)
    wc = np.asarray(inputs["Wc"], dtype=np.float32).astype(ml_dtypes.bfloat16)
    wcls = np.asarray(inputs["Wcls"], dtype=np.float32)
    bf = np.asarray(inputs["bf"], dtype=np.float32)
    ba = np.asarray(inputs["ba"], dtype=np.float32)
    bbh = np.asarray(inputs["bb"], dtype=np.float32) * 0.5
    bc = np.asarray(inputs["bc"], dtype=np.float32).reshape([1])
    bcls = np.asarray(inputs["bcls"], dtype=np.float32)

    shared = {
        "wf": wf, "wa": wa, "wb": wb, "wc": wc, "wcls": wcls,
        "bf": bf, "ba": ba, "bbh": bbh, "bc": bc, "bcls": bcls,
    }
    in_maps = []
    for c in range(CORES):
        shard = np.ascontiguousarray(x[:, c * NS:(c + 1) * NS, :])
        in_maps.append({"x": shard, **shared})
    return in_maps


def kernel(**inputs) -> np.ndarray:
    nc = _get_nc()
    in_maps = make_in_maps(inputs)
    res = run_bass_kernel_spmd(nc, in_maps, core_ids=list(range(CORES)))
    return np.asarray(res.results[0]["out"], dtype=np.float32)


if __name__ == "__main__":
    nc = build_kernel()
    print("build OK, instructions:", len(nc.m.functions[0].instructions)
          if hasattr(nc.m.functions[0], "instructions") else "?")
